# revision 6
# baseline (speedup 1.0000x reference)
"""Multi-head self-attention Trainium2 kernel (8 NeuronCores, batch-parallel).

Problem: x[8,1024,768], w_qkv[768,2304], w_proj[768,768], b_proj[768]
Returns (out[8,1024,768], attn[8,12,1024,1024]) matching the eager reference.

Each core handles one batch element. Per core:
  xT        = x^T (PE transpose)
  qkT       = w_qk^T @ xT        [1536,1024]  (q/k features on partitions)
  v         = x @ w_v            [1024,768] -> bf16, with a ones column per
              head block (65-wide blocks) so the AV matmul also produces
              softmax row-sums.
  per head h:
    scoresT[m,n] = k_h @ q_h^T -> exp (ScalarE, scale=1/8) -> bf16 expT
    out_unnorm[n,64],rowsum[n] = expT^T-contracted with v block (bf16 matmul)
    scores[n,m]  = q_h @ k_h^T -> exp (fp32) -> * (1/rowsum) -> attn out DMA
    out_cat[n, h*64:..] = out_unnorm * (1/rowsum)
  proj: final[n,e] = out_cat @ w_proj + b_proj (bias as K=1 matmul accum)
"""

import sys

for _p in ("/opt/trn_rl_repo", "/root/.axon_site/_ro/trn_rl_repo"):
    if _p not in sys.path:
        sys.path.append(_p)

import numpy as np

import concourse.bass as bass
import concourse.mybir as mybir
import concourse.tile as tile
from concourse import bacc
from concourse import bass_utils
from concourse.masks import make_identity

dt = mybir.dt
AF = mybir.ActivationFunctionType
ALU = mybir.AluOpType

B = 8
N = 1024
E = 768
H = 12
D = 64
SCALE = float(D) ** -0.5  # 0.125
NT = N // 128  # 8 seq tiles
ET = E // 128  # 6 feature tiles
F32 = dt.float32
F32R = dt.float32r
BF16 = dt.bfloat16

VW = D + 1  # v block width per head incl. ones column -> 65


def r32(ap):
    return ap.bitcast(F32R)


def build_nc():
    nc = bacc.Bacc("TRN2", target_bir_lowering=False, debug=False)

    x_d = nc.dram_tensor("x", [N, E], F32, kind="ExternalInput")
    wqkv_d = nc.dram_tensor("w_qkv", [E, 3 * E], F32, kind="ExternalInput")
    wproj_d = nc.dram_tensor("w_proj", [E, E], F32, kind="ExternalInput")
    b_d = nc.dram_tensor("b_proj", [E], F32, kind="ExternalInput")
    out_d = nc.dram_tensor("out", [N, E], F32, kind="ExternalOutput")
    attn_d = nc.dram_tensor("attn", [H, N, N], F32, kind="ExternalOutput")

    with tile.TileContext(nc) as tc:
        with (
            # ---- persistent SBUF ----
            tc.tile_pool(name="const", bufs=1) as constp,
            tc.tile_pool(name="qkT", bufs=1) as qkTp,
            tc.tile_pool(name="vbf", bufs=1) as vbfp,
            tc.tile_pool(name="ocat", bufs=1) as ocatp,
            tc.tile_pool(name="recip", bufs=32) as recipp,
        ):
            ident = constp.tile([128, 128], F32, tag="ident")
            make_identity(nc, ident[:])
            qkT = qkTp.tile([128, 12 * N], F32R, tag="qkT")  # 48KB/part
            vbf = vbfp.tile([128, NT * H * VW], BF16, tag="vbf")  # 12.2KB/part
            out_cat = ocatp.tile([128, NT * E], F32, tag="ocat")  # 24KB/part

            # =========== phase 1: load + transpose x, qkv matmuls ==========
            with (
                tc.tile_pool(name="p1", bufs=1) as p1,
                tc.tile_pool(name="p1ps", bufs=4, space="PSUM") as p1ps,
                tc.tile_pool(name="qvps", bufs=2, space="PSUM") as qvps,
            ):
                wq = p1.tile([128, ET * 3 * E], F32R, tag="wq")  # 55KB/part
                for i in range(ET):
                    nc.scalar.dma_start(
                        wq[:, i * 3 * E : (i + 1) * 3 * E],
                        wqkv_d[i * 128 : (i + 1) * 128, :].bitcast(F32R),
                    )
                xin = p1.tile([128, NT * E], F32, tag="xin")  # 24KB/part
                for t in range(NT):
                    nc.scalar.dma_start(
                        xin[:, t * E : (t + 1) * E], x_d[t * 128 : (t + 1) * 128, :]
                    )
                xT = p1.tile([128, ET * N], F32R, tag="xT")  # 24KB/part
                for t in range(NT):
                    for i in range(ET):
                        ps = p1ps.tile([128, 128], F32, tag="tps")
                        nc.tensor.transpose(
                            ps[:], xin[:, t * E + i * 128 : t * E + (i + 1) * 128],
                            ident[:],
                        )
                        nc.vector.tensor_copy(
                            xT[:, i * N + t * 128 : i * N + (t + 1) * 128], ps[:]
                        )

                # qkT[j*128+p, n] for j in 0..11 (q rows then k rows)
                for j in range(12):
                    for c in range(2):  # n chunks of 512
                        psq = qvps.tile([128, 512], F32, tag="psq")
                        for i in range(ET):
                            nc.tensor.matmul(
                                psq[:],
                                r32(wq[:, i * 3 * E + j * 128 : i * 3 * E + (j + 1) * 128]),
                                r32(xT[:, i * N + c * 512 : i * N + (c + 1) * 512]),
                                start=(i == 0),
                                stop=(i == ET - 1),
                            )
                        nc.vector.tensor_copy(
                            qkT[:, j * N + c * 512 : j * N + (c + 1) * 512], psq[:]
                        )

                # v = x @ w_v  (w_v = cols 1536:2304), written into 65-wide
                # per-head blocks of vbf (bf16) + ones column per head.
                for t in range(NT):
                    base = t * H * VW
                    for c, w in ((0, 512), (512, 256)):
                        psv = qvps.tile([128, 512], F32, tag="psv")
                        for i in range(ET):
                            nc.tensor.matmul(
                                psv[:, :w],
                                r32(xT[:, i * N + t * 128 : i * N + (t + 1) * 128]),
                                r32(wq[:, i * 3 * E + 1536 + c : i * 3 * E + 1536 + c + w]),
                                start=(i == 0),
                                stop=(i == ET - 1),
                            )
                        nh = w // D  # heads in this chunk
                        h0 = c // D
                        dst = vbf[:, base + h0 * VW :].rearrange(
                            "p (h v) -> p h v", v=VW
                        )[:, :nh, :D]
                        src = psv[:, :w].rearrange("p (h v) -> p h v", v=D)
                        nc.vector.tensor_copy(dst, src)
                    ones = vbf[:, base:].rearrange("p (h v) -> p h v", v=VW)[
                        :, :H, D : D + 1
                    ]
                    nc.gpsimd.memset(ones, 1.0)

            # ================== phase 2: per-head attention ==================
            with (
                tc.tile_pool(name="expT", bufs=2) as expTp,
                tc.tile_pool(name="attn_u", bufs=3) as attnup,
                tc.tile_pool(name="attn_f", bufs=3) as attnfp,
                tc.tile_pool(name="psT", bufs=2, space="PSUM") as psTp,
                tc.tile_pool(name="psS", bufs=1, space="PSUM") as psSp,
                tc.tile_pool(name="psA", bufs=2, space="PSUM") as psAp,
            ):
                for h in range(H):
                    po = (h % 2) * 64  # partition offset of this head's q/k rows
                    qc = (h // 2) * N  # col offset of q tile in qkT
                    kc = (6 + h // 2) * N  # col offset of k tile in qkT

                    expT = expTp.tile([128, NT * N], BF16, tag="expT")  # 16KB
                    # ---- scoresT[m,n] = k q^T, exp -> bf16 ----
                    for mt in range(NT):
                        psT = psTp.tile([128, N], F32, tag="psT")
                        for c in range(2):
                            nc.tensor.matmul(
                                psT[:, c * 512 : (c + 1) * 512],
                                r32(qkT[po : po + 64, kc + mt * 128 : kc + (mt + 1) * 128]),
                                r32(qkT[po : po + 64, qc + c * 512 : qc + (c + 1) * 512]),
                                start=True,
                                stop=True,
                            )
                        nc.scalar.activation(
                            expT[:, mt * N : (mt + 1) * N], psT[:], AF.Exp, scale=SCALE
                        )

                    # ---- AV: out_unnorm[n,64] + rowsum[n] ----
                    rc = []
                    for t in range(NT):
                        psa = psAp.tile([128, VW], F32, tag="psa")
                        for mt in range(NT):
                            nc.tensor.matmul(
                                psa[:],
                                expT[:, mt * N + t * 128 : mt * N + (t + 1) * 128],
                                vbf[:, mt * H * VW + h * VW : mt * H * VW + (h + 1) * VW],
                                start=(mt == 0),
                                stop=(mt == NT - 1),
                            )
                        rec = recipp.tile([128, 1], F32, tag="rec")
                        nc.vector.reciprocal(rec[:], psa[:, D : D + 1])
                        rc.append(rec)
                        nc.vector.tensor_scalar(
                            out_cat[:, t * E + h * D : t * E + (h + 1) * D],
                            psa[:, :D],
                            rec[:],
                            None,
                            ALU.mult,
                        )

                    # ---- scores[n,m], exp fp32, normalize, DMA out ----
                    for t in range(NT):
                        psS = psSp.tile([128, N], F32, tag="psS")
                        for c in range(2):
                            nc.tensor.matmul(
                                psS[:, c * 512 : (c + 1) * 512],
                                r32(qkT[po : po + 64, qc + t * 128 : qc + (t + 1) * 128]),
                                r32(qkT[po : po + 64, kc + c * 512 : kc + (c + 1) * 512]),
                                start=True,
                                stop=True,
                            )
                        a_u = attnup.tile([128, N], F32, tag="a_u")
                        nc.scalar.activation(a_u[:], psS[:], AF.Exp, scale=SCALE)
                        a_f = attnfp.tile([128, N], F32, tag="a_f")
                        nc.vector.tensor_scalar(
                            a_f[:], a_u[:], rc[t][:], None, ALU.mult
                        )
                        nc.sync.dma_start(
                            attn_d[h, t * 128 : (t + 1) * 128, :], a_f[:]
                        )

            # ==================== phase 3: output proj ====================
            with (
                tc.tile_pool(name="p3", bufs=1) as p3,
                tc.tile_pool(name="outf", bufs=2) as outfp,
                tc.tile_pool(name="p3ps", bufs=4, space="PSUM") as p3ps,
                tc.tile_pool(name="prps", bufs=2, space="PSUM") as prps,
            ):
                wp = p3.tile([128, ET * E], F32R, tag="wp")  # 18KB/part
                for i in range(ET):
                    nc.scalar.dma_start(
                        wp[:, i * E : (i + 1) * E], wproj_d[i * 128 : (i + 1) * 128, :].bitcast(F32R)
                    )
                b_row = p3.tile([1, E], F32R, tag="b_row")
                nc.scalar.dma_start(b_row[:], b_d[None, :].bitcast(F32R))
                ones_row32 = p3.tile([1, 128], F32, tag="ones_row32")
                nc.gpsimd.memset(ones_row32[:], 1.0)
                ones_row = p3.tile([1, 128], F32R, tag="ones_row")
                nc.vector.tensor_copy(ones_row[:], ones_row32[:])

                ocT = p3.tile([128, ET * N], F32R, tag="ocT")  # 24KB/part
                for t in range(NT):
                    for j in range(ET):
                        ps = p3ps.tile([128, 128], F32, tag="tps3")
                        nc.tensor.transpose(
                            ps[:], out_cat[:, t * E + j * 128 : t * E + (j + 1) * 128],
                            ident[:],
                        )
                        nc.vector.tensor_copy(
                            ocT[:, j * N + t * 128 : j * N + (t + 1) * 128], ps[:]
                        )

                for t in range(NT):
                    psf = prps.tile([128, E], F32, tag="psf")
                    for c, w in ((0, 512), (512, 256)):
                        for j in range(ET):
                            nc.tensor.matmul(
                                psf[:, c : c + w],
                                r32(ocT[:, j * N + t * 128 : j * N + (t + 1) * 128]),
                                r32(wp[:, j * E + c : j * E + c + w]),
                                start=(j == 0),
                                stop=False,
                            )
                        nc.tensor.matmul(
                            psf[:, c : c + w],
                            r32(ones_row[:]),
                            r32(b_row[:, c : c + w]),
                            start=False,
                            stop=True,
                        )
                    of = outfp.tile([128, E], F32, tag="of")
                    nc.vector.tensor_copy(of[:], psf[:])
                    nc.sync.dma_start(out_d[t * 128 : (t + 1) * 128, :], of[:])

    nc.compile()
    return nc


_NC = None


def get_nc():
    global _NC
    if _NC is None:
        _NC = build_nc()
    return _NC


def kernel(x, w_qkv, w_proj, b_proj):
    nc = get_nc()
    x = np.ascontiguousarray(np.asarray(x, dtype=np.float32))
    w_qkv = np.ascontiguousarray(np.asarray(w_qkv, dtype=np.float32))
    w_proj = np.ascontiguousarray(np.asarray(w_proj, dtype=np.float32))
    b_proj = np.ascontiguousarray(np.asarray(b_proj, dtype=np.float32))
    in_maps = [
        {"x": x[c], "w_qkv": w_qkv, "w_proj": w_proj, "b_proj": b_proj}
        for c in range(B)
    ]
    res = bass_utils.run_bass_kernel_spmd(nc, in_maps, core_ids=list(range(B)))
    out = np.stack([res.results[c]["out"] for c in range(B)])
    attn = np.stack([res.results[c]["attn"] for c in range(B)])
    return out, attn


# revision 22
# speedup vs baseline: 1.2450x; 1.2450x over previous
"""Multi-head self-attention Trainium2 kernel (8 NeuronCores, batch-parallel).

Problem: x[8,1024,768], w_qkv[768,2304], w_proj[768,768], b_proj[768]
Returns (out[8,1024,768], attn[8,12,1024,1024]) matching the eager reference.

Each core handles one batch element. Per core:
  xT        = x^T (PE transpose)
  qkT       = w_qk^T @ xT        [1536,1024]  (q/k features on partitions)
  v         = x @ w_v            [1024,768] -> bf16, with a ones column per
              head block (65-wide blocks) so the AV matmul also produces
              softmax row-sums.
  per head h:
    scoresT[m,n] = k_h @ q_h^T -> exp (ScalarE, scale=1/8) -> bf16 expT
    out_unnorm[n,64],rowsum[n] = expT^T-contracted with v block (bf16 matmul)
    scores[n,m]  = q_h @ k_h^T -> exp (fp32) -> * (1/rowsum) -> attn out DMA
    out_cat[n, h*64:..] = out_unnorm * (1/rowsum)
  proj: final[n,e] = out_cat @ w_proj + b_proj (bias as K=1 matmul accum)
"""

import sys

for _p in ("/opt/trn_rl_repo", "/root/.axon_site/_ro/trn_rl_repo"):
    if _p not in sys.path:
        sys.path.append(_p)

import numpy as np

import concourse.bass as bass
import concourse.mybir as mybir
import concourse.tile as tile
from concourse import bacc
from concourse import bass_utils
from concourse.masks import make_identity

dt = mybir.dt
AF = mybir.ActivationFunctionType
ALU = mybir.AluOpType

B = 8
N = 1024
E = 768
H = 12
D = 64
SCALE = float(D) ** -0.5  # 0.125
NT = N // 128  # 8 seq tiles
ET = E // 128  # 6 feature tiles
F32 = dt.float32
F32R = dt.float32r
BF16 = dt.bfloat16

VW = D + 1  # v block width per head incl. ones column -> 65


def r32(ap):
    return ap.bitcast(F32R)


def build_nc():
    nc = bacc.Bacc("TRN2", target_bir_lowering=False, debug=False)

    x_d = nc.dram_tensor("x", [N, E], F32, kind="ExternalInput")
    wqkv_d = nc.dram_tensor("w_qkv", [E, 3 * E], F32, kind="ExternalInput")
    wproj_d = nc.dram_tensor("w_proj", [E, E], F32, kind="ExternalInput")
    b_d = nc.dram_tensor("b_proj", [E], F32, kind="ExternalInput")
    out_d = nc.dram_tensor("out", [N, E], F32, kind="ExternalOutput")
    attn_d = nc.dram_tensor("attn", [H, N, N], F32, kind="ExternalOutput")

    with tile.TileContext(nc) as tc:
        with (
            # ---- persistent SBUF ----
            tc.tile_pool(name="const", bufs=1) as constp,
            tc.tile_pool(name="qkT", bufs=1) as qkTp,
            tc.tile_pool(name="vbf", bufs=1) as vbfp,
        ):
            ident = constp.tile([128, 128], F32, tag="ident")
            make_identity(nc, ident[:])
            # per-head q/k tiles [128,1024]: head h occupies partition rows
            # (h%2)*64..+64, the other 64 rows are zero (lets every score
            # matmul run at K=128 full-array speed; the zero half of the
            # stationary operand kills the other head's rows in the moving
            # operand).
            q2T = qkTp.tile([128, H * N], F32R, tag="q2T")  # 48KB/part
            k2T = qkTp.tile([128, H * N], F32R, tag="k2T")  # 48KB/part
            vbf = vbfp.tile([128, NT * H * VW], BF16, tag="vbf")  # 12.2KB/part

            # =========== phase 1: load + transpose x, qkv matmuls ==========
            with (
                tc.tile_pool(name="p1", bufs=1) as p1,
                tc.tile_pool(name="p1x", bufs=2) as p1x,
                tc.tile_pool(name="p1ps", bufs=2, space="PSUM") as p1ps,
                tc.tile_pool(name="qvps", bufs=2, space="PSUM") as qvps,
            ):
                xT = p1.tile([128, ET * N], F32R, tag="xT")  # 24KB/part
                for t in range(NT):
                    xin = p1x.tile([128, E], F32, tag="xin")  # streamed, bufs via tag
                    nc.scalar.dma_start(xin[:], x_d[t * 128 : (t + 1) * 128, :])
                    for i in range(ET):
                        ps = p1ps.tile([128, 128], F32, tag="tps")
                        nc.tensor.transpose(
                            ps[:], xin[:, i * 128 : (i + 1) * 128], ident[:]
                        )
                        nc.vector.tensor_copy(
                            xT[:, i * N + t * 128 : i * N + (t + 1) * 128], ps[:]
                        )
                wq = p1.tile([128, ET * 3 * E], F32R, tag="wq")  # 55KB/part
                for i in range(ET):
                    nc.scalar.dma_start(
                        wq[:, i * 3 * E : (i + 1) * 3 * E],
                        wqkv_d[i * 128 : (i + 1) * 128, :].bitcast(F32R),
                    )

                # parity masks: even-head rows 0..63 = 1 else 0, odd inverse
                mask_e = p1.tile([128, 1], F32, tag="mask_e")
                nc.gpsimd.memset(mask_e[0:64, :], 1.0)
                nc.gpsimd.memset(mask_e[64:128, :], 0.0)
                mask_o = p1.tile([128, 1], F32, tag="mask_o")
                nc.gpsimd.memset(mask_o[0:64, :], 0.0)
                nc.gpsimd.memset(mask_o[64:128, :], 1.0)
                # qkv matmuls: j in 0..5 are q row-tiles (heads 2j,2j+1),
                # j in 6..11 are k row-tiles. Masked copies zero-fill the
                # opposite parity half of each per-head tile.
                for j in (0, 6, 1, 7, 2, 8, 3, 9, 4, 10, 5, 11):
                    dstt = q2T if j < 6 else k2T
                    hb = (j % 6) * 2  # first head in this row-tile
                    for c in range(2):  # n chunks of 512
                        psq = qvps.tile([128, 512], F32, tag="psq")
                        for i in range(ET):
                            nc.tensor.matmul(
                                psq[:],
                                r32(wq[:, i * 3 * E + j * 128 : i * 3 * E + (j + 1) * 128]),
                                r32(xT[:, i * N + c * 512 : i * N + (c + 1) * 512]),
                                start=(i == 0),
                                stop=(i == ET - 1),
                            )
                        nc.vector.tensor_scalar(
                            dstt[:, hb * N + c * 512 : hb * N + (c + 1) * 512],
                            psq[:],
                            mask_e[:],
                            None,
                            ALU.mult,
                        )
                        nc.vector.tensor_scalar(
                            dstt[:, (hb + 1) * N + c * 512 : (hb + 1) * N + (c + 1) * 512],
                            psq[:],
                            mask_o[:],
                            None,
                            ALU.mult,
                        )

                # v = x @ w_v  (w_v = cols 1536:2304), written into 65-wide
                # per-head blocks of vbf (bf16) + ones column per head.
                for t in range(NT):
                    base = t * H * VW
                    for c, w in ((0, 512), (512, 256)):
                        psv = qvps.tile([128, 512], F32, tag="psv")
                        for i in range(ET):
                            nc.tensor.matmul(
                                psv[:, :w],
                                r32(xT[:, i * N + t * 128 : i * N + (t + 1) * 128]),
                                r32(wq[:, i * 3 * E + 1536 + c : i * 3 * E + 1536 + c + w]),
                                start=(i == 0),
                                stop=(i == ET - 1),
                            )
                        nh = w // D  # heads in this chunk
                        h0 = c // D
                        dst = vbf[:, base + h0 * VW :].rearrange(
                            "p (h v) -> p h v", v=VW
                        )[:, :nh, :D]
                        src = psv[:, :w].rearrange("p (h v) -> p h v", v=D)
                        nc.vector.tensor_copy(dst, src)
                    ones = vbf[:, base:].rearrange("p (h v) -> p h v", v=VW)[
                        :, :H, D : D + 1
                    ]
                    nc.gpsimd.memset(ones, 1.0)

            # ================== phase 2: per-head attention ==================
            with (
                tc.tile_pool(name="ocat", bufs=1) as ocatp,
                tc.tile_pool(name="recip", bufs=32) as recipp,
            ):
              out_cat = ocatp.tile([128, NT * E], F32, tag="ocat")  # 24KB/part
              with (
                tc.tile_pool(name="expT", bufs=2) as expTp,
                tc.tile_pool(name="attn_u", bufs=3) as attnup,
                tc.tile_pool(name="attn_f", bufs=3) as attnfp,
                tc.tile_pool(name="ps2", bufs=3, space="PSUM") as ps2p,
                tc.tile_pool(name="psA", bufs=2, space="PSUM") as psAp,
              ):
                recs = {}  # (h, t) -> recip tile

                def passB_unit(h, mt, expT):
                    """scoresT[mtile, :] matmuls + exp -> bf16 expT slice."""
                    psT = ps2p.tile([128, N], F32, tag="ps2")
                    for c in range(2):
                        nc.tensor.matmul(
                            psT[:, c * 512 : (c + 1) * 512],
                            k2T[:, h * N + mt * 128 : h * N + (mt + 1) * 128],
                            q2T[:, h * N + c * 512 : h * N + (c + 1) * 512],
                            start=True,
                            stop=True,
                        )
                    nc.scalar.activation(
                        expT[:, mt * N : (mt + 1) * N], psT[:], AF.Exp, scale=SCALE
                    )

                def passA_unit(h, t):
                    """scores[ntile, :] matmuls + exp + normalize + DMA."""
                    psS = ps2p.tile([128, N], F32, tag="ps2")
                    for c in range(2):
                        nc.tensor.matmul(
                            psS[:, c * 512 : (c + 1) * 512],
                            q2T[:, h * N + t * 128 : h * N + (t + 1) * 128],
                            k2T[:, h * N + c * 512 : h * N + (c + 1) * 512],
                            start=True,
                            stop=True,
                        )
                    a_u = attnup.tile([128, N], F32, tag="a_u")
                    nc.scalar.activation(a_u[:], psS[:], AF.Exp, scale=SCALE)
                    a_f = attnfp.tile([128, N], F32, tag="a_f")
                    nc.vector.tensor_scalar(
                        a_f[:], a_u[:], recs.pop((h, t))[:], None, ALU.mult
                    )
                    nc.sync.dma_start(attn_d[h, t * 128 : (t + 1) * 128, :], a_f[:])

                def av_unit(h, t, expT):
                    """AV accumulation for ntile t + rowsum/recip + out_cat."""
                    psa = psAp.tile([128, VW], F32, tag="psa")
                    for mt in range(NT):
                        nc.tensor.matmul(
                            psa[:],
                            expT[:, mt * N + t * 128 : mt * N + (t + 1) * 128],
                            vbf[:, mt * H * VW + h * VW : mt * H * VW + (h + 1) * VW],
                            start=(mt == 0),
                            stop=(mt == NT - 1),
                        )
                    rec = recipp.tile([128, 1], F32, tag="rec")
                    nc.vector.reciprocal(rec[:], psa[:, D : D + 1])
                    recs[(h, t)] = rec
                    nc.vector.tensor_scalar(
                        out_cat[:, t * E + h * D : t * E + (h + 1) * D],
                        psa[:, :D],
                        rec[:],
                        None,
                        ALU.mult,
                    )

                # software pipeline: head h's scoresT/exp interleaves with
                # head h-1's scores/exp/normalize/DMA (whose rowsums exist).
                expTs = {}
                for h in range(H):
                    expT = expTp.tile([128, NT * N], BF16, tag="expT")
                    expTs[h] = expT
                    for u in range(NT):
                        passB_unit(h, u, expT)
                        if h > 0:
                            passA_unit(h - 1, u)
                    for t in range(NT):
                        av_unit(h, t, expT)
                    expTs.pop(h - 1, None)
                for u in range(NT):
                    passA_unit(H - 1, u)

            # ==================== phase 3: output proj ====================
            with (
                tc.tile_pool(name="p3", bufs=1) as p3,
                tc.tile_pool(name="outf", bufs=2) as outfp,
                tc.tile_pool(name="p3ps", bufs=4, space="PSUM") as p3ps,
                tc.tile_pool(name="prps", bufs=2, space="PSUM") as prps,
            ):
                wp = p3.tile([128, ET * E], F32R, tag="wp")  # 18KB/part
                for i in range(ET):
                    nc.scalar.dma_start(
                        wp[:, i * E : (i + 1) * E], wproj_d[i * 128 : (i + 1) * 128, :].bitcast(F32R)
                    )
                b_row = p3.tile([1, E], F32R, tag="b_row")
                nc.scalar.dma_start(b_row[:], b_d[None, :].bitcast(F32R))
                ones_row32 = p3.tile([1, 128], F32, tag="ones_row32")
                nc.gpsimd.memset(ones_row32[:], 1.0)
                ones_row = p3.tile([1, 128], F32R, tag="ones_row")
                nc.vector.tensor_copy(ones_row[:], ones_row32[:])

                ocT = p3.tile([128, ET * N], F32R, tag="ocT")  # 24KB/part
                for t in range(NT):
                    for j in range(ET):
                        ps = p3ps.tile([128, 128], F32, tag="tps3")
                        nc.tensor.transpose(
                            ps[:], out_cat[:, t * E + j * 128 : t * E + (j + 1) * 128],
                            ident[:],
                        )
                        nc.vector.tensor_copy(
                            ocT[:, j * N + t * 128 : j * N + (t + 1) * 128], ps[:]
                        )

                for t in range(NT):
                    psf = prps.tile([128, E], F32, tag="psf")
                    for c, w in ((0, 512), (512, 256)):
                        for j in range(ET):
                            nc.tensor.matmul(
                                psf[:, c : c + w],
                                r32(ocT[:, j * N + t * 128 : j * N + (t + 1) * 128]),
                                r32(wp[:, j * E + c : j * E + c + w]),
                                start=(j == 0),
                                stop=False,
                            )
                        nc.tensor.matmul(
                            psf[:, c : c + w],
                            r32(ones_row[:]),
                            r32(b_row[:, c : c + w]),
                            start=False,
                            stop=True,
                        )
                    of = outfp.tile([128, E], F32, tag="of")
                    nc.vector.tensor_copy(of[:], psf[:])
                    nc.sync.dma_start(out_d[t * 128 : (t + 1) * 128, :], of[:])

    nc.compile()
    return nc


_NC = None


def get_nc():
    global _NC
    if _NC is None:
        _NC = build_nc()
    return _NC


def kernel(x, w_qkv, w_proj, b_proj):
    nc = get_nc()
    x = np.ascontiguousarray(np.asarray(x, dtype=np.float32))
    w_qkv = np.ascontiguousarray(np.asarray(w_qkv, dtype=np.float32))
    w_proj = np.ascontiguousarray(np.asarray(w_proj, dtype=np.float32))
    b_proj = np.ascontiguousarray(np.asarray(b_proj, dtype=np.float32))
    in_maps = [
        {"x": x[c], "w_qkv": w_qkv, "w_proj": w_proj, "b_proj": b_proj}
        for c in range(B)
    ]
    res = bass_utils.run_bass_kernel_spmd(nc, in_maps, core_ids=list(range(B)))
    out = np.stack([res.results[c]["out"] for c in range(B)])
    attn = np.stack([res.results[c]["attn"] for c in range(B)])
    return out, attn
